# revision 8
# baseline (speedup 1.0000x reference)
"""Trainium2 Bass kernel for the patch-correlation + softmax + flow-regression module.

Math: for each batch, match[k,q] = sum_{s in 3x3} <f2n[k+s], f1n[q+s]> where f1n/f2n are
channel-L2-normalized features. flow = softmax_k(10*match) regressed against source coords.

Kernel strategy (per core = one (batch, query-half)):
  - k laid out padded: k' = ki*50 + kj (kj in [0,50), cols 48/49 zero). 24 chunks of 100 rows
    (2 image rows per chunk) so +-1 diagonal shifts never cross useful chunk boundaries.
  - The 3 row-shifts (s1) of the 3x3 patch sum fold into 3 PSUM-accumulated matmuls with
    column-shifted (by 50*s1) operands from zero-guarded feature buffers.
  - The 3 col-shifts (s2) become 2 vector adds per chunk (diag +-1 reads of V; zero pad
    columns make all boundary terms vanish).
  - softmax+regression: out rows (sum E*ki, sum E*kj, sum E) via one 3-column matmul over
    E = exp(match) (x10 folded into f2 normalization; no max-subtraction needed — softmax
    ratio is shift-invariant and values are small for normalized features).
  - Final division + coordinate subtraction on host (tiny: 3x2304 per batch).
"""

import math

import numpy as np

import concourse.bacc as bacc
import concourse.mybir as mybir
import concourse.tile as tile
from concourse.bass_utils import run_bass_kernel_spmd

F32 = mybir.dt.float32
AF = mybir.ActivationFunctionType

H = W = 48
C = 256
HW = H * W
WP = 50              # padded image-row width
KP = H * WP          # 2400 padded k extent
GK = 64              # zero guard cols on each side of feature buffers
QWIN = 26            # f1 window image rows (24 + 1 halo each side)
F1W = QWIN * WP      # 1300
NCH = 19             # k chunks of 128 rows in padded-k space (last has 96)
NBLK = 3             # q blocks per core
QB = 8 * WP          # padded cols per q block (8 image rows)

N_CORES = 8
_CACHE = {}

LAST_EXEC_NS = None
TRACE = False


def _build_nc():
    nc = bacc.Bacc("TRN2", target_bir_lowering=False, debug=False, num_devices=N_CORES)

    f2_in = nc.dram_tensor("f2", [C, KP], F32, kind="ExternalInput")
    f1_in = nc.dram_tensor("f1", [C, F1W], F32, kind="ExternalInput")
    wsw_in = nc.dram_tensor("wsw", [128, 3 * NCH], F32, kind="ExternalInput")
    out_dram = nc.dram_tensor("out", [3, NBLK * QB], F32, kind="ExternalOutput")

    with tile.TileContext(nc) as tc:
        with (
            tc.tile_pool(name="const", bufs=1) as const_pool,
            tc.tile_pool(name="fbuf", bufs=1) as fbuf_pool,
            tc.tile_pool(name="sq", bufs=3) as sq_pool,
            tc.tile_pool(name="inv", bufs=2) as inv_pool,
            tc.tile_pool(name="match", bufs=3) as match_pool,
            tc.tile_pool(name="vps", bufs=2, space="PSUM") as v_psum,
            tc.tile_pool(name="wsps", bufs=2, space="PSUM") as ws_psum,
            tc.tile_pool(name="n2ps", bufs=2, space="PSUM") as n2_psum,
            tc.tile_pool(name="bcps", bufs=2, space="PSUM") as bc_psum,
        ):
            ones = const_pool.tile([128, 128], F32)
            nc.vector.memset(ones[:, :], 1.0)
            eps_t = const_pool.tile([1, 1], F32)
            nc.vector.memset(eps_t[:, :], 1e-12)
            log10_t = const_pool.tile([1, 1], F32)
            nc.vector.memset(log10_t[:, :], math.log(10.0))
            wsw_t = const_pool.tile([128, 3 * NCH], F32)
            nc.sync.dma_start(out=wsw_t[:, :], in_=wsw_in[:, :])
            outb = const_pool.tile([3, NBLK * QB], F32)

            f2t = [fbuf_pool.tile([128, GK + KP + GK], F32, name=f"f2t{cc}", tag=f"f2t{cc}") for cc in range(2)]
            f1t = [fbuf_pool.tile([128, GK + F1W + GK], F32, name=f"f1t{cc}", tag=f"f1t{cc}") for cc in range(2)]

            for tiles, wreal, src in ((f2t, KP, f2_in), (f1t, F1W, f1_in)):
                for cc in range(2):
                    t = tiles[cc]
                    nc.vector.memset(t[:, 0:GK], 0.0)
                    nc.vector.memset(t[:, GK + wreal:GK + wreal + GK], 0.0)
                    nc.sync.dma_start(
                        out=t[:, GK:GK + wreal],
                        in_=src[cc * 128:(cc + 1) * 128, :],
                    )

            def normalize(ft, wreal, bias_ap):
                # scale columns of ft by exp(-0.5*ln(n2 + eps) + logscale) = e^logscale/|col|
                o = 0
                while o < wreal:
                    T = min(480, wreal - o)
                    n2 = n2_psum.tile([1, 512], F32, name="n2", tag="n2")
                    for cc in range(2):
                        sq = sq_pool.tile([128, 512], F32, name="sq", tag="sq")
                        nc.scalar.activation(sq[:, 0:T], ft[cc][:, GK + o:GK + o + T], AF.Square)
                        nc.tensor.matmul(
                            n2[:, 0:T], lhsT=ones[:, 0:1], rhs=sq[:, 0:T],
                            start=(cc == 0), stop=(cc == 1),
                        )
                    lnt = inv_pool.tile([1, 512], F32, name="lnt", tag="lnt")
                    nc.scalar.activation(lnt[0:1, 0:T], n2[0:1, 0:T], AF.Ln, bias=eps_t[0:1, 0:1])
                    invn = inv_pool.tile([1, 512], F32, name="invn", tag="invn")
                    nc.scalar.activation(invn[0:1, 0:T], lnt[0:1, 0:T], AF.Exp,
                                         scale=-0.5, bias=bias_ap)
                    bc = bc_psum.tile([128, 512], F32, name="bc", tag="bc")
                    nc.tensor.matmul(bc[:, 0:T], lhsT=ones[0:1, :], rhs=invn[0:1, 0:T],
                                     start=True, stop=True)
                    for cc in range(2):
                        nc.vector.tensor_mul(
                            ft[cc][:, GK + o:GK + o + T],
                            ft[cc][:, GK + o:GK + o + T],
                            bc[:, 0:T],
                        )
                    o += T

            normalize(f2t, KP, log10_t[0:1, 0:1])   # fold softmax scale into f2
            normalize(f1t, F1W, 0.0)  # bias 0.0 has a registered const AP

            deltas = [50 * s1 + s2 for s1 in (-1, 0, 1) for s2 in (-1, 0, 1)]
            for j in range(NBLK):
                q0 = (1 + 8 * j) * WP
                wsps = ws_psum.tile([3, QB], F32, name="wsps", tag="wsps")
                for c in range(NCH):
                    rows = min(128, KP - 128 * c)
                    V = v_psum.tile([128, QB], F32, name="V", tag="V")
                    k = 0
                    for d in deltas:
                        for cc in range(2):
                            nc.tensor.matmul(
                                V[0:rows, :],
                                lhsT=f2t[cc][:, GK + 128 * c + d:
                                             GK + 128 * c + d + rows],
                                rhs=f1t[cc][:, GK + q0 + d:GK + q0 + d + QB],
                                start=(k == 0), stop=(k == 17),
                            )
                            k += 1
                    m = match_pool.tile([128, QB], F32, name="m", tag="m")
                    nc.scalar.activation(m[0:rows, :], V[0:rows, :], AF.Exp)
                    nc.tensor.matmul(
                        wsps[:, :], lhsT=wsw_t[0:rows, 3 * c:3 * c + 3], rhs=m[0:rows, :],
                        start=(c == 0), stop=(c == NCH - 1),
                    )
                nc.scalar.copy(out=outb[:, QB * j:QB * (j + 1)], in_=wsps[:, :])
            nc.sync.dma_start(out=out_dram[:, :], in_=outb[:, :])

    nc.compile()
    return nc


def _pad_rows(x2d):
    # [C, R*48] -> [C, R*50] zero-padding cols 48,49 of each image row
    rows = x2d.shape[1] // W
    out = np.zeros((x2d.shape[0], rows * WP), np.float32)
    out.reshape(x2d.shape[0], rows, WP)[:, :, :W] = x2d.reshape(x2d.shape[0], rows, W)
    return out


def _ws_weights():
    wsw = np.zeros((128, 3 * NCH), np.float32)
    for c in range(NCH):
        kp = 128 * c + np.arange(128)
        ki, kj = kp // WP, kp % WP
        valid = (kp < KP) & (kj < 48)
        wsw[:, 3 * c + 0] = np.where(valid, ki.astype(np.float32), 0.0)
        wsw[:, 3 * c + 1] = np.where(valid, kj.astype(np.float32), 0.0)
        wsw[:, 3 * c + 2] = np.where(valid, 1.0, 0.0)
    return wsw


def _maybe_enable_trace():
    """Register the axon NTFF profiling hook if available (test-time only)."""
    try:
        import sys
        import types
        if "antenv.axon_hooks" not in sys.modules:
            mod = types.ModuleType("antenv.axon_hooks")
            holder = [None]
            mod.set_axon_ntff_profile_hook = lambda h: holder.__setitem__(0, h)
            mod.get_axon_ntff_profile_hook = lambda: holder[0]
            sys.modules["antenv.axon_hooks"] = mod
        from trn_agent_boot.trn_boot import _ntff_profile_via_ctypes
        sys.modules["antenv.axon_hooks"].set_axon_ntff_profile_hook(
            _ntff_profile_via_ctypes("/opt/axon/libaxon_pjrt.so")
        )
        return True
    except Exception:
        return False


def kernel(feature_1, feature_2):
    global LAST_EXEC_NS
    f1 = np.asarray(feature_1, dtype=np.float32)
    f2 = np.asarray(feature_2, dtype=np.float32)
    B = f1.shape[0]
    assert f1.shape == (B, C, H, W) and f2.shape == (B, C, H, W)

    if "nc" not in _CACHE:
        _CACHE["nc"] = _build_nc()
    nc = _CACHE["nc"]

    wsw = _ws_weights()
    in_maps = []
    for core in range(N_CORES):
        b, half = divmod(core, 2)
        b = b % B
        f2pad = _pad_rows(f2[b].reshape(C, HW))
        qi0 = 24 * half
        win = np.zeros((C, QWIN, W), np.float32)
        lo = max(0, qi0 - 1)
        hi = min(H, qi0 + QWIN - 1)
        win[:, lo - (qi0 - 1):hi - (qi0 - 1)] = f1[b].reshape(C, H, W)[:, lo:hi]
        f1win = _pad_rows(win.reshape(C, QWIN * W))
        in_maps.append({"f2": f2pad, "f1": f1win, "wsw": wsw})

    trace = TRACE and _maybe_enable_trace()
    res = run_bass_kernel_spmd(nc, in_maps, list(range(N_CORES)), trace=trace)
    LAST_EXEC_NS = res.exec_time_ns

    out = np.zeros((B, 2, H, W), np.float32)
    qj = np.arange(W, dtype=np.float32)[None, :]
    for core in range(N_CORES):
        b, half = divmod(core, 2)
        b = b % B
        o = np.asarray(res.results[core]["out"]).reshape(3, QROWS_ := 24, WP)[:, :, :W]
        eh = o[0] / o[2]
        ew = o[1] / o[2]
        qi0 = 24 * half
        qi = (qi0 + np.arange(QROWS_, dtype=np.float32))[:, None]
        out[b, 0, qi0:qi0 + QROWS_] = ew - qj
        out[b, 1, qi0:qi0 + QROWS_] = eh - qi
    return out


# revision 10
# speedup vs baseline: 2.8019x; 2.8019x over previous
"""Trainium2 Bass kernel for the patch-correlation + softmax + flow-regression module.

Math: for each batch, match[k,q] = sum_{s in 3x3} <f2n[k+s], f1n[q+s]> where f1n/f2n are
channel-L2-normalized features. flow = softmax_k(10*match) regressed against source coords.

Kernel strategy (per core = one (batch, query-half)):
  - k laid out padded: k' = ki*50 + kj (kj in [0,50), cols 48/49 zero). 24 chunks of 100 rows
    (2 image rows per chunk) so +-1 diagonal shifts never cross useful chunk boundaries.
  - The 3 row-shifts (s1) of the 3x3 patch sum fold into 3 PSUM-accumulated matmuls with
    column-shifted (by 50*s1) operands from zero-guarded feature buffers.
  - The 3 col-shifts (s2) become 2 vector adds per chunk (diag +-1 reads of V; zero pad
    columns make all boundary terms vanish).
  - softmax+regression: out rows (sum E*ki, sum E*kj, sum E) via one 3-column matmul over
    E = exp(match) (x10 folded into f2 normalization; no max-subtraction needed — softmax
    ratio is shift-invariant and values are small for normalized features).
  - Final division + coordinate subtraction on host (tiny: 3x2304 per batch).
"""

import math

import numpy as np

import concourse.bacc as bacc
import concourse.mybir as mybir
import concourse.tile as tile
from concourse.bass_utils import run_bass_kernel_spmd

F32 = mybir.dt.float32
BF16 = mybir.dt.bfloat16
AF = mybir.ActivationFunctionType

H = W = 48
C = 256
HW = H * W
WP = 50              # padded image-row width
KP = H * WP          # 2400 padded k extent
GK = 64              # zero guard cols on each side of feature buffers
QWIN = 26            # f1 window image rows (24 + 1 halo each side)
F1W = QWIN * WP      # 1300
NCH = 19             # k chunks of 128 rows in padded-k space (last has 96)
NBLK = 3             # q blocks per core
QB = 8 * WP          # padded cols per q block (8 image rows)

N_CORES = 8
_CACHE = {}

LAST_EXEC_NS = None
TRACE = False


def _build_nc():
    nc = bacc.Bacc("TRN2", target_bir_lowering=False, debug=False, num_devices=N_CORES)

    f2_in = nc.dram_tensor("f2", [C, KP], F32, kind="ExternalInput")
    f1_in = nc.dram_tensor("f1", [C, F1W], F32, kind="ExternalInput")
    wsw_in = nc.dram_tensor("wsw", [128, 3 * NCH], F32, kind="ExternalInput")
    out_dram = nc.dram_tensor("out", [3, NBLK * QB], F32, kind="ExternalOutput")

    with tile.TileContext(nc) as tc:
        with (
            tc.tile_pool(name="const", bufs=1) as const_pool,
            tc.tile_pool(name="fbuf", bufs=1) as fbuf_pool,
            tc.tile_pool(name="sq", bufs=3) as sq_pool,
            tc.tile_pool(name="inv", bufs=2) as inv_pool,
            tc.tile_pool(name="match", bufs=3) as match_pool,
            tc.tile_pool(name="vps", bufs=2, space="PSUM") as v_psum,
            tc.tile_pool(name="wsps", bufs=2, space="PSUM") as ws_psum,
            tc.tile_pool(name="n2ps", bufs=2, space="PSUM") as n2_psum,
            tc.tile_pool(name="bcps", bufs=2, space="PSUM") as bc_psum,
        ):
            ones = const_pool.tile([128, 128], F32)
            nc.vector.memset(ones[:, :], 1.0)
            eps_t = const_pool.tile([1, 1], F32)
            nc.vector.memset(eps_t[:, :], 1e-12)
            log10_t = const_pool.tile([1, 1], F32)
            nc.vector.memset(log10_t[:, :], math.log(10.0))
            wsw_t = const_pool.tile([128, 3 * NCH], F32)
            nc.sync.dma_start(out=wsw_t[:, :], in_=wsw_in[:, :])
            outb = const_pool.tile([3, NBLK * QB], F32)

            f2t = [fbuf_pool.tile([128, GK + KP + GK], F32, name=f"f2t{cc}", tag=f"f2t{cc}") for cc in range(2)]
            f1t = [fbuf_pool.tile([128, GK + F1W + GK], F32, name=f"f1t{cc}", tag=f"f1t{cc}") for cc in range(2)]
            # bf16 copies of the normalized features feed the big correlation
            # matmuls (fp32 PE matmul runs as 2 passes = half throughput).
            f2b = [fbuf_pool.tile([128, GK + KP + GK], BF16, name=f"f2b{cc}", tag=f"f2b{cc}") for cc in range(2)]
            f1b = [fbuf_pool.tile([128, GK + F1W + GK], BF16, name=f"f1b{cc}", tag=f"f1b{cc}") for cc in range(2)]

            for tiles, btiles, wreal, src in ((f2t, f2b, KP, f2_in), (f1t, f1b, F1W, f1_in)):
                for cc in range(2):
                    t = tiles[cc]
                    nc.sync.dma_start(
                        out=t[:, GK:GK + wreal],
                        in_=src[cc * 128:(cc + 1) * 128, :],
                    )
                    b = btiles[cc]
                    nc.vector.memset(b[:, 0:GK], 0.0)
                    nc.vector.memset(b[:, GK + wreal:GK + wreal + GK], 0.0)

            def normalize(ft, fb, wreal, bias_ap):
                # scale columns of ft by exp(-0.5*ln(n2 + eps) + logscale) = e^logscale/|col|
                o = 0
                while o < wreal:
                    T = min(480, wreal - o)
                    n2 = n2_psum.tile([1, 512], F32, name="n2", tag="n2")
                    for cc in range(2):
                        sq = sq_pool.tile([128, 512], F32, name="sq", tag="sq")
                        nc.scalar.activation(sq[:, 0:T], ft[cc][:, GK + o:GK + o + T], AF.Square)
                        nc.tensor.matmul(
                            n2[:, 0:T], lhsT=ones[:, 0:1], rhs=sq[:, 0:T],
                            start=(cc == 0), stop=(cc == 1),
                        )
                    lnt = inv_pool.tile([1, 512], F32, name="lnt", tag="lnt")
                    nc.scalar.activation(lnt[0:1, 0:T], n2[0:1, 0:T], AF.Ln, bias=eps_t[0:1, 0:1])
                    invn = inv_pool.tile([1, 512], F32, name="invn", tag="invn")
                    nc.scalar.activation(invn[0:1, 0:T], lnt[0:1, 0:T], AF.Exp,
                                         scale=-0.5, bias=bias_ap)
                    bc = bc_psum.tile([128, 512], F32, name="bc", tag="bc")
                    nc.tensor.matmul(bc[:, 0:T], lhsT=ones[0:1, :], rhs=invn[0:1, 0:T],
                                     start=True, stop=True)
                    for cc in range(2):
                        nc.vector.tensor_mul(
                            fb[cc][:, GK + o:GK + o + T],
                            ft[cc][:, GK + o:GK + o + T],
                            bc[:, 0:T],
                        )
                    o += T

            normalize(f2t, f2b, KP, log10_t[0:1, 0:1])   # fold softmax scale into f2
            normalize(f1t, f1b, F1W, 0.0)  # bias 0.0 has a registered const AP

            deltas = [50 * s1 + s2 for s1 in (-1, 0, 1) for s2 in (-1, 0, 1)]
            for j in range(NBLK):
                q0 = (1 + 8 * j) * WP
                wsps = ws_psum.tile([3, QB], F32, name="wsps", tag="wsps")
                for c in range(NCH):
                    rows = min(128, KP - 128 * c)
                    V = v_psum.tile([128, QB], F32, name="V", tag="V")
                    k = 0
                    for d in deltas:
                        for cc in range(2):
                            nc.tensor.matmul(
                                V[0:rows, :],
                                lhsT=f2b[cc][:, GK + 128 * c + d:
                                             GK + 128 * c + d + rows],
                                rhs=f1b[cc][:, GK + q0 + d:GK + q0 + d + QB],
                                start=(k == 0), stop=(k == 17),
                            )
                            k += 1
                    m = match_pool.tile([128, QB], F32, name="m", tag="m")
                    nc.scalar.activation(m[0:rows, :], V[0:rows, :], AF.Exp)
                    nc.tensor.matmul(
                        wsps[:, :], lhsT=wsw_t[0:rows, 3 * c:3 * c + 3], rhs=m[0:rows, :],
                        start=(c == 0), stop=(c == NCH - 1),
                    )
                nc.scalar.copy(out=outb[:, QB * j:QB * (j + 1)], in_=wsps[:, :])
            nc.sync.dma_start(out=out_dram[:, :], in_=outb[:, :])

    nc.compile()
    return nc


def _pad_rows(x2d):
    # [C, R*48] -> [C, R*50] zero-padding cols 48,49 of each image row
    rows = x2d.shape[1] // W
    out = np.zeros((x2d.shape[0], rows * WP), np.float32)
    out.reshape(x2d.shape[0], rows, WP)[:, :, :W] = x2d.reshape(x2d.shape[0], rows, W)
    return out


def _ws_weights():
    wsw = np.zeros((128, 3 * NCH), np.float32)
    for c in range(NCH):
        kp = 128 * c + np.arange(128)
        ki, kj = kp // WP, kp % WP
        valid = (kp < KP) & (kj < 48)
        wsw[:, 3 * c + 0] = np.where(valid, ki.astype(np.float32), 0.0)
        wsw[:, 3 * c + 1] = np.where(valid, kj.astype(np.float32), 0.0)
        wsw[:, 3 * c + 2] = np.where(valid, 1.0, 0.0)
    return wsw


def _maybe_enable_trace():
    """Register the axon NTFF profiling hook if available (test-time only)."""
    try:
        import sys
        import types
        if "antenv.axon_hooks" not in sys.modules:
            mod = types.ModuleType("antenv.axon_hooks")
            holder = [None]
            mod.set_axon_ntff_profile_hook = lambda h: holder.__setitem__(0, h)
            mod.get_axon_ntff_profile_hook = lambda: holder[0]
            sys.modules["antenv.axon_hooks"] = mod
        from trn_agent_boot.trn_boot import _ntff_profile_via_ctypes
        sys.modules["antenv.axon_hooks"].set_axon_ntff_profile_hook(
            _ntff_profile_via_ctypes("/opt/axon/libaxon_pjrt.so")
        )
        return True
    except Exception:
        return False


def kernel(feature_1, feature_2):
    global LAST_EXEC_NS
    f1 = np.asarray(feature_1, dtype=np.float32)
    f2 = np.asarray(feature_2, dtype=np.float32)
    B = f1.shape[0]
    assert f1.shape == (B, C, H, W) and f2.shape == (B, C, H, W)

    if "nc" not in _CACHE:
        _CACHE["nc"] = _build_nc()
    nc = _CACHE["nc"]

    wsw = _ws_weights()
    in_maps = []
    for core in range(N_CORES):
        b, half = divmod(core, 2)
        b = b % B
        f2pad = _pad_rows(f2[b].reshape(C, HW))
        qi0 = 24 * half
        win = np.zeros((C, QWIN, W), np.float32)
        lo = max(0, qi0 - 1)
        hi = min(H, qi0 + QWIN - 1)
        win[:, lo - (qi0 - 1):hi - (qi0 - 1)] = f1[b].reshape(C, H, W)[:, lo:hi]
        f1win = _pad_rows(win.reshape(C, QWIN * W))
        in_maps.append({"f2": f2pad, "f1": f1win, "wsw": wsw})

    trace = TRACE and _maybe_enable_trace()
    res = run_bass_kernel_spmd(nc, in_maps, list(range(N_CORES)), trace=trace)
    LAST_EXEC_NS = res.exec_time_ns

    out = np.zeros((B, 2, H, W), np.float32)
    qj = np.arange(W, dtype=np.float32)[None, :]
    for core in range(N_CORES):
        b, half = divmod(core, 2)
        b = b % B
        o = np.asarray(res.results[core]["out"]).reshape(3, QROWS_ := 24, WP)[:, :, :W]
        eh = o[0] / o[2]
        ew = o[1] / o[2]
        qi0 = 24 * half
        qi = (qi0 + np.arange(QROWS_, dtype=np.float32))[:, None]
        out[b, 0, qi0:qi0 + QROWS_] = ew - qj
        out[b, 1, qi0:qi0 + QROWS_] = eh - qi
    return out


# revision 12
# speedup vs baseline: 3.5011x; 1.2495x over previous
"""Trainium2 Bass kernel for the patch-correlation + softmax + flow-regression module.

Math: for each batch, match[k,q] = sum_{s in 3x3} <f2n[k+s], f1n[q+s]> where f1n/f2n are
channel-L2-normalized features. flow = softmax_k(10*match) regressed against source coords.

Kernel strategy (per core = one (batch, query-half)):
  - k laid out padded: k' = ki*50 + kj (kj in [0,50), cols 48/49 zero). 24 chunks of 100 rows
    (2 image rows per chunk) so +-1 diagonal shifts never cross useful chunk boundaries.
  - The 3 row-shifts (s1) of the 3x3 patch sum fold into 3 PSUM-accumulated matmuls with
    column-shifted (by 50*s1) operands from zero-guarded feature buffers.
  - The 3 col-shifts (s2) become 2 vector adds per chunk (diag +-1 reads of V; zero pad
    columns make all boundary terms vanish).
  - softmax+regression: out rows (sum E*ki, sum E*kj, sum E) via one 3-column matmul over
    E = exp(match) (x10 folded into f2 normalization; no max-subtraction needed — softmax
    ratio is shift-invariant and values are small for normalized features).
  - Final division + coordinate subtraction on host (tiny: 3x2304 per batch).
"""

import math

import numpy as np

import concourse.bacc as bacc
import concourse.mybir as mybir
import concourse.tile as tile
from concourse.bass_utils import run_bass_kernel_spmd

F32 = mybir.dt.float32
BF16 = mybir.dt.bfloat16
AF = mybir.ActivationFunctionType

H = W = 48
C = 256
HW = H * W
WP = 50              # padded image-row width
KP = H * WP          # 2400 padded k extent
GK = 64              # zero guard cols on each side of feature buffers
QWIN = 26            # f1 window image rows (24 + 1 halo each side)
F1W = QWIN * WP      # 1300
NCH = 24             # k chunks of 100 rows (2 image rows each)
SDT = mybir.dt.float32   # dtype of the diag-shift pipeline (vs/vsp/vsm/m)
NBLK = 3             # q blocks per core
QB = 8 * WP          # padded cols per q block (8 image rows)

N_CORES = 8
_CACHE = {}

LAST_EXEC_NS = None
TRACE = False


def _build_nc():
    nc = bacc.Bacc("TRN2", target_bir_lowering=False, debug=False, num_devices=N_CORES)

    f2_in = nc.dram_tensor("f2", [C, KP], F32, kind="ExternalInput")
    f1_in = nc.dram_tensor("f1", [C, F1W], F32, kind="ExternalInput")
    wsw_in = nc.dram_tensor("wsw", [128, 3 * NCH], F32, kind="ExternalInput")
    out_dram = nc.dram_tensor("out", [3, NBLK * QB], F32, kind="ExternalOutput")

    with tile.TileContext(nc) as tc:
        with (
            tc.tile_pool(name="const", bufs=1) as const_pool,
            tc.tile_pool(name="fbuf", bufs=1) as fbuf_pool,
            tc.tile_pool(name="sq", bufs=3) as sq_pool,
            tc.tile_pool(name="inv", bufs=2) as inv_pool,
            tc.tile_pool(name="match", bufs=3) as match_pool,
            tc.tile_pool(name="vps", bufs=2, space="PSUM") as v_psum,
            tc.tile_pool(name="wsps", bufs=2, space="PSUM") as ws_psum,
            tc.tile_pool(name="n2ps", bufs=2, space="PSUM") as n2_psum,
            tc.tile_pool(name="bcps", bufs=2, space="PSUM") as bc_psum,
        ):
            ones = const_pool.tile([128, 128], F32)
            nc.vector.memset(ones[:, :], 1.0)
            eps_t = const_pool.tile([1, 1], F32)
            nc.vector.memset(eps_t[:, :], 1e-12)
            log10_t = const_pool.tile([1, 1], F32)
            nc.vector.memset(log10_t[:, :], math.log(10.0))
            wsw_t = const_pool.tile([128, 3 * NCH], F32)
            nc.sync.dma_start(out=wsw_t[:, :], in_=wsw_in[:, :])
            outb = const_pool.tile([3, NBLK * QB], F32)

            f2t = [fbuf_pool.tile([128, GK + KP + GK], F32, name=f"f2t{cc}", tag=f"f2t{cc}") for cc in range(2)]
            f1t = [fbuf_pool.tile([128, GK + F1W + GK], F32, name=f"f1t{cc}", tag=f"f1t{cc}") for cc in range(2)]
            # bf16 copies of the normalized features feed the big correlation
            # matmuls (fp32 PE matmul runs as 2 passes = half throughput).
            f2b = [fbuf_pool.tile([128, GK + KP + GK], BF16, name=f"f2b{cc}", tag=f"f2b{cc}") for cc in range(2)]
            f1b = [fbuf_pool.tile([128, GK + F1W + GK], BF16, name=f"f1b{cc}", tag=f"f1b{cc}") for cc in range(2)]

            for tiles, btiles, wreal, src in ((f2t, f2b, KP, f2_in), (f1t, f1b, F1W, f1_in)):
                for cc in range(2):
                    t = tiles[cc]
                    nc.sync.dma_start(
                        out=t[:, GK:GK + wreal],
                        in_=src[cc * 128:(cc + 1) * 128, :],
                    )
                    b = btiles[cc]
                    nc.vector.memset(b[:, 0:GK], 0.0)
                    nc.vector.memset(b[:, GK + wreal:GK + wreal + GK], 0.0)

            def normalize(ft, fb, wreal, bias_ap):
                # scale columns of ft by exp(-0.5*ln(n2 + eps) + logscale) = e^logscale/|col|
                o = 0
                while o < wreal:
                    T = min(480, wreal - o)
                    n2 = n2_psum.tile([1, 512], F32, name="n2", tag="n2")
                    for cc in range(2):
                        sq = sq_pool.tile([128, 512], F32, name="sq", tag="sq")
                        nc.scalar.activation(sq[:, 0:T], ft[cc][:, GK + o:GK + o + T], AF.Square)
                        nc.tensor.matmul(
                            n2[:, 0:T], lhsT=ones[:, 0:1], rhs=sq[:, 0:T],
                            start=(cc == 0), stop=(cc == 1),
                        )
                    lnt = inv_pool.tile([1, 512], F32, name="lnt", tag="lnt")
                    nc.scalar.activation(lnt[0:1, 0:T], n2[0:1, 0:T], AF.Ln, bias=eps_t[0:1, 0:1])
                    invn = inv_pool.tile([1, 512], F32, name="invn", tag="invn")
                    nc.scalar.activation(invn[0:1, 0:T], lnt[0:1, 0:T], AF.Exp,
                                         scale=-0.5, bias=bias_ap)
                    bc = bc_psum.tile([128, 512], F32, name="bc", tag="bc")
                    nc.tensor.matmul(bc[:, 0:T], lhsT=ones[0:1, :], rhs=invn[0:1, 0:T],
                                     start=True, stop=True)
                    for cc in range(2):
                        nc.vector.tensor_mul(
                            fb[cc][:, GK + o:GK + o + T],
                            ft[cc][:, GK + o:GK + o + T],
                            bc[:, 0:T],
                        )
                    o += T

            normalize(f2t, f2b, KP, log10_t[0:1, 0:1])   # fold softmax scale into f2
            normalize(f1t, f1b, F1W, 0.0)  # bias 0.0 has a registered const AP

            # Main loop: chunks of 100 k'-rows (2 image rows, so chunk-boundary
            # rows are kj=49 zero-pads and +-1 diag shifts never need data from a
            # neighboring chunk). Per chunk:
            #   V[p, jv] = sum_s1 C[k'(p)+50*s1, q'(jv)+50*s1]  (6 bf16 matmuls, PSUM)
            #   diag terms V[p+1, jv+1] / V[p-1, jv-1] materialized by DMA
            #   partition-shifted copies (compute engines require quadrant-aligned
            #   partition windows; DMA is the only engine that can shift partitions).
            for j in range(NBLK):
                q0 = (1 + 8 * j) * WP
                wsps = ws_psum.tile([3, QB], F32, name="wsps", tag="wsps")
                for c in range(NCH):
                    V = v_psum.tile([128, QB + 2], F32, name="V", tag="V")
                    k = 0
                    for s1 in (-1, 0, 1):
                        for cc in range(2):
                            nc.tensor.matmul(
                                V[0:101, :],
                                lhsT=f2b[cc][:, GK + 100 * c + 50 * s1:
                                             GK + 100 * c + 50 * s1 + 101],
                                rhs=f1b[cc][:, GK + q0 - 1 + 50 * s1:
                                            GK + q0 - 1 + 50 * s1 + QB + 2],
                                start=(k == 0), stop=(k == 5),
                            )
                            k += 1
                    vs = match_pool.tile([128, QB + 2], SDT, name="vs", tag="vs")
                    nc.scalar.copy(out=vs[0:101, :], in_=V[0:101, :])
                    vsp = match_pool.tile([128, QB], SDT, name="vsp", tag="vsp")
                    nc.sync.dma_start(out=vsp[0:100, :], in_=vs[1:101, 2:QB + 2])
                    vsm = match_pool.tile([128, QB], SDT, name="vsm", tag="vsm")
                    nc.gpsimd.memset(vsm[0:1, :], 0.0)
                    nc.sync.dma_start(out=vsm[1:100, :], in_=vs[0:99, 0:QB])
                    m = match_pool.tile([128, QB], SDT, name="m", tag="m")
                    nc.vector.tensor_add(m[0:100, :], vs[0:100, 1:QB + 1], vsp[0:100, :])
                    nc.vector.tensor_add(m[0:100, :], m[0:100, :], vsm[0:100, :])
                    me = match_pool.tile([128, QB], F32, name="me", tag="me")
                    nc.scalar.activation(me[0:100, :], m[0:100, :], AF.Exp)
                    nc.tensor.matmul(
                        wsps[:, :], lhsT=wsw_t[0:100, 3 * c:3 * c + 3], rhs=me[0:100, :],
                        start=(c == 0), stop=(c == NCH - 1),
                    )
                nc.scalar.copy(out=outb[:, QB * j:QB * (j + 1)], in_=wsps[:, :])
            nc.sync.dma_start(out=out_dram[:, :], in_=outb[:, :])

    nc.compile()
    return nc


def _pad_rows(x2d):
    # [C, R*48] -> [C, R*50] zero-padding cols 48,49 of each image row
    rows = x2d.shape[1] // W
    out = np.zeros((x2d.shape[0], rows * WP), np.float32)
    out.reshape(x2d.shape[0], rows, WP)[:, :, :W] = x2d.reshape(x2d.shape[0], rows, W)
    return out


def _ws_weights():
    wsw = np.zeros((128, 3 * NCH), np.float32)
    for c in range(NCH):
        kp = 100 * c + np.arange(128)
        ki, kj = kp // WP, kp % WP
        valid = (kp < KP) & (kj < 48) & (np.arange(128) < 100)
        wsw[:, 3 * c + 0] = np.where(valid, ki.astype(np.float32), 0.0)
        wsw[:, 3 * c + 1] = np.where(valid, kj.astype(np.float32), 0.0)
        wsw[:, 3 * c + 2] = np.where(valid, 1.0, 0.0)
    return wsw


def _maybe_enable_trace():
    """Register the axon NTFF profiling hook if available (test-time only)."""
    try:
        import sys
        import types
        if "antenv.axon_hooks" not in sys.modules:
            mod = types.ModuleType("antenv.axon_hooks")
            holder = [None]
            mod.set_axon_ntff_profile_hook = lambda h: holder.__setitem__(0, h)
            mod.get_axon_ntff_profile_hook = lambda: holder[0]
            sys.modules["antenv.axon_hooks"] = mod
        from trn_agent_boot.trn_boot import _ntff_profile_via_ctypes
        sys.modules["antenv.axon_hooks"].set_axon_ntff_profile_hook(
            _ntff_profile_via_ctypes("/opt/axon/libaxon_pjrt.so")
        )
        return True
    except Exception:
        return False


def kernel(feature_1, feature_2):
    global LAST_EXEC_NS
    f1 = np.asarray(feature_1, dtype=np.float32)
    f2 = np.asarray(feature_2, dtype=np.float32)
    B = f1.shape[0]
    assert f1.shape == (B, C, H, W) and f2.shape == (B, C, H, W)

    if "nc" not in _CACHE:
        _CACHE["nc"] = _build_nc()
    nc = _CACHE["nc"]

    wsw = _ws_weights()
    in_maps = []
    for core in range(N_CORES):
        b, half = divmod(core, 2)
        b = b % B
        f2pad = _pad_rows(f2[b].reshape(C, HW))
        qi0 = 24 * half
        win = np.zeros((C, QWIN, W), np.float32)
        lo = max(0, qi0 - 1)
        hi = min(H, qi0 + QWIN - 1)
        win[:, lo - (qi0 - 1):hi - (qi0 - 1)] = f1[b].reshape(C, H, W)[:, lo:hi]
        f1win = _pad_rows(win.reshape(C, QWIN * W))
        in_maps.append({"f2": f2pad, "f1": f1win, "wsw": wsw})

    trace = TRACE and _maybe_enable_trace()
    res = run_bass_kernel_spmd(nc, in_maps, list(range(N_CORES)), trace=trace)
    LAST_EXEC_NS = res.exec_time_ns

    out = np.zeros((B, 2, H, W), np.float32)
    qj = np.arange(W, dtype=np.float32)[None, :]
    for core in range(N_CORES):
        b, half = divmod(core, 2)
        b = b % B
        o = np.asarray(res.results[core]["out"]).reshape(3, QROWS_ := 24, WP)[:, :, :W]
        eh = o[0] / o[2]
        ew = o[1] / o[2]
        qi0 = 24 * half
        qi = (qi0 + np.arange(QROWS_, dtype=np.float32))[:, None]
        out[b, 0, qi0:qi0 + QROWS_] = ew - qj
        out[b, 1, qi0:qi0 + QROWS_] = eh - qi
    return out
